# revision 2
# baseline (speedup 1.0000x reference)
"""DTW loss kernel for Trainium2 (Bass) — hardware-loop version.

Computes sqrt(DTW^2(source, target)) for source, target of shape (2048,) via
    D[i,j] = (s_i - t_j)^2 + min(D[i-1,j], D[i,j-1], D[i-1,j-1])

Same wavefront mapping as the unrolled baseline (128 column-chunks of 16
columns; partition p computes DP row r = t - 2p at wavefront step t; one DP
row-chunk = one vector-engine tensor_tensor_scan over 32 interleaved slots),
but the 2302 steps run inside For_i hardware loops so the static program is
~60 instructions instead of ~9000 (per-call dispatch cost scales with static
program size here).

Changes vs the unrolled baseline:
- Cross-chunk boundary via one SBUF->SBUF DMA per step (partition p's last
  column -> partition p+1's halo slot) instead of PE shift-matmul + scalar
  bias fix. Partition 0's halo slot is never written and stays INF.
- Cost ring holds a full half-wavefront phase (1152 steps x 32 slots), filled
  by 16 scalar-engine Square activations per phase before each loop; no
  refills inside the loop.
- The DP corner (DTW[0,0]=0) is realized by poking ring slot (t=0, cell 0,
  even) to -1e30: state = min(INF, INF) + (-1e30) = 0, then the odd slot adds
  the real cost on top. The slot is re-zeroed between phases.
- sdiag (source diagonally shifted per partition) is built on-device by one
  DMA with a per-partition +2 element stride over a padded copy of source.
  To keep the partition step positive (negative steps are rejected by the BIR
  verifier), partition p owns column block 127-p: block b's wavefront delay
  2b maps to partition p = 127-b, halos flow from partition p+1 to p, and the
  final DP cell lives on partition 0.
"""

import os
import sys

for _p in ("/opt/trn_rl_repo", "/root/.axon_site/_ro/trn_rl_repo"):
    if os.path.isdir(_p) and _p not in sys.path:
        sys.path.insert(0, _p)

import numpy as np

import concourse.bass as bass
import concourse.bacc as bacc
import concourse.mybir as mybir
import concourse.tile as tile
from concourse.bass import ds
from concourse.bass_utils import run_bass_kernel_spmd

F32 = mybir.dt.float32

N = 2048            # sequence length (both source and target)
P = 128             # partitions / column chunks
CW = N // P         # columns per chunk (16)
SW = 2 * CW + 2     # strip width: [halo | 2*CW scan slots | pad]
SLACK = 2           # wavefront steps of slack per chunk
T = N + SLACK * (P - 1)   # 2302 total wavefront steps
PH1 = 1152                # phase-1 steps (even)
PH2 = T - PH1             # phase-2 steps (1150, even)
M = T + 2                 # sdiag columns (padded)
PADF = 2 * (P - 1) + 2    # front padding of s_pad (256)
SPAD = PADF + M           # s_pad length
INF = np.float32(1e30)
PAD = np.float32(1e15)    # sdiag pad; squares to 1e30

_cache = {}


def _build():
    nc = bacc.Bacc("TRN2", target_bir_lowering=False, debug=False)

    s_pad = nc.dram_tensor("s_pad", [1, SPAD], F32, kind="ExternalInput")
    negt = nc.dram_tensor("negt", [P, CW], F32, kind="ExternalInput")
    res = nc.dram_tensor("res", [1, 1], F32, kind="ExternalOutput")

    with tile.TileContext(nc) as tc:
        with tc.tile_pool(name="sb", bufs=1) as pool:
            t_sdiag = pool.tile([P, M], F32)
            t_negt = pool.tile([P, CW], F32)
            t_ring = pool.tile([P, PH1 * 2 * CW], F32)
            t_sa = pool.tile([P, SW], F32)
            t_sb = pool.tile([P, SW], F32)

            # Partition p owns column block 127-p (delay 2*(127-p)):
            # sdiag[p, m] = s_pad[PADF - 2*(P-1) + 2p + m] = s[m - 2*(127-p)]
            sd_src = bass.AP(s_pad, PADF - SLACK * (P - 1), [[SLACK, P], [1, M]])
            nc.sync.dma_start(t_sdiag[:], sd_src)
            nc.sync.dma_start(t_negt[:], negt[:])

            # zeros in the even (d1) slots persist for the whole run
            nc.gpsimd.memset(t_ring[:], 0.0)
            nc.vector.memset(t_sa[:], float(INF))
            nc.vector.memset(t_sb[:], float(INF))

            strips = [t_sa, t_sb]
            eng = nc.vector
            pstr = int(t_sa.ap[0][0])
            rstr = int(t_ring.ap[0][0])

            def produce(phase_base, cnt):
                # ring[p, step*2CW + 2j+1] = (sdiag[p, phase_base+step] - t_j)^2
                for j in range(CW):
                    out_ap = bass.AP(
                        t_ring.tensor,
                        t_ring.offset + 2 * j + 1,
                        [[rstr, P], [2 * CW, cnt]],
                    )
                    nc.scalar.activation(
                        out_ap,
                        t_sdiag[:, phase_base : phase_base + cnt],
                        mybir.ActivationFunctionType.Square,
                        bias=t_negt[:, j : j + 1],
                        scale=1.0,
                    )

            def scan_step(cur, prev, d1_ap):
                d0 = bass.AP(
                    prev.tensor, prev.offset + 2, [[pstr, P], [2, CW], [-2, 2]]
                )
                eng.add_instruction(
                    mybir.InstTensorScalarPtr(
                        name=nc.get_next_instruction_name(),
                        is_tensor_tensor_scan=True,
                        is_scalar_tensor_tensor=True,
                        op0=mybir.AluOpType.min,
                        op1=mybir.AluOpType.add,
                        ins=[
                            eng.lower_ap(d0),
                            eng.lower_ap(cur[:, 0:1]),
                            eng.lower_ap(d1_ap),
                        ],
                        outs=[eng.lower_ap(cur[:, 1 : 2 * CW + 1])],
                    )
                )

            def halo_dma(strip):
                # block b's halo comes from block b-1 = partition p+1:
                # strip[p+1, 2CW] -> strip[p, 0] for p = 0..P-2
                nc.sync.dma_start(
                    strip[0 : P - 1, 0:1], strip[1:P, 2 * CW : 2 * CW + 1]
                )

            def phase(nsteps):
                # The halo from scan t is shipped during step t+1 (one step of
                # delay), so it lands after scan t+1 consumed the previous halo
                # as its upleft value and before scan t+2 reads it as initial.
                with tc.For_i(0, nsteps // 2, 1) as i:
                    scan_step(t_sa, t_sb, t_ring[:, ds(i * (4 * CW), 2 * CW)])
                    halo_dma(t_sb)
                    scan_step(
                        t_sb, t_sa, t_ring[:, ds(i * (4 * CW) + 2 * CW, 2 * CW)]
                    )
                    halo_dma(t_sa)

            # ---- phase 1: steps 0 .. PH1-1
            produce(0, PH1)
            # DP corner: DTW[0,0]=0 enters via ring slot (t=0, cell 0, even):
            # state = min(INF, INF) + (-1e30) = 0, then the odd slot adds the
            # real cost. Broadcast to all partitions (single-partition APs at
            # p=127 are rejected): on blocks b>=1 step 0 is a garbage row
            # whose ~1e30 cost re-swamps the zeroed state, so it's harmless.
            nc.vector.memset(t_ring[:, 0:1], -float(INF))
            phase(PH1)

            # ---- phase 2: steps PH1 .. T-1
            nc.vector.memset(t_ring[:, 0:1], 0.0)
            produce(PH1, PH2)
            phase(PH2)

            # final DP cell: block 127 = partition 0, at step T-1
            final = strips[(T - 1) % 2]
            nc.sync.dma_start(res[0:1, 0:1], final[0:1, 2 * CW : 2 * CW + 1])
    nc.compile()
    return nc


def _prep_inputs(source, target):
    source = np.asarray(source, np.float32).reshape(N)
    target = np.asarray(target, np.float32).reshape(N)
    sp = np.full((1, SPAD), PAD, np.float32)
    sp[0, PADF : PADF + N] = source
    # partition p owns column block 127-p
    negt = (-target.reshape(P, CW)[::-1]).astype(np.float32).copy()
    return {"s_pad": sp, "negt": negt}


def _run(inputs, trace=False):
    if "nc" not in _cache:
        _cache["nc"] = _build()
    nc = _cache["nc"]
    r = run_bass_kernel_spmd(
        nc, [dict(inputs) for _ in range(8)], core_ids=list(range(8)), trace=trace
    )
    return r


def kernel(source, target):
    inputs = _prep_inputs(source, target)
    r = _run(inputs)
    loss_sq = r.results[0]["res"][0, 0]
    return np.sqrt(np.float32(loss_sq))[None].astype(np.float32)


# revision 3
# speedup vs baseline: 1.4051x; 1.4051x over previous
"""DTW loss kernel for Trainium2 (Bass) — hardware-loop version.

Computes sqrt(DTW^2(source, target)) for source, target of shape (2048,) via
    D[i,j] = (s_i - t_j)^2 + min(D[i-1,j], D[i,j-1], D[i-1,j-1])

Same wavefront mapping as the unrolled baseline (128 column-chunks of 16
columns; partition p computes DP row r = t - 2p at wavefront step t; one DP
row-chunk = one vector-engine tensor_tensor_scan over 32 interleaved slots),
but the 2302 steps run inside For_i hardware loops so the static program is
~60 instructions instead of ~9000 (per-call dispatch cost scales with static
program size here).

Changes vs the unrolled baseline:
- Cross-chunk boundary via one SBUF->SBUF DMA per step (partition p's last
  column -> partition p+1's halo slot) instead of PE shift-matmul + scalar
  bias fix. Partition 0's halo slot is never written and stays INF.
- Cost ring holds a full half-wavefront phase (1152 steps x 32 slots), filled
  by 16 scalar-engine Square activations per phase before each loop; no
  refills inside the loop.
- The DP corner (DTW[0,0]=0) is realized by poking ring slot (t=0, cell 0,
  even) to -1e30: state = min(INF, INF) + (-1e30) = 0, then the odd slot adds
  the real cost on top. The slot is re-zeroed between phases.
- sdiag (source diagonally shifted per partition) is built on-device by one
  DMA with a per-partition +2 element stride over a padded copy of source.
  To keep the partition step positive (negative steps are rejected by the BIR
  verifier), partition p owns column block 127-p: block b's wavefront delay
  2b maps to partition p = 127-b, halos flow from partition p+1 to p, and the
  final DP cell lives on partition 0.
"""

import os
import sys

for _p in ("/opt/trn_rl_repo", "/root/.axon_site/_ro/trn_rl_repo"):
    if os.path.isdir(_p) and _p not in sys.path:
        sys.path.insert(0, _p)

import numpy as np

import concourse.bass as bass
import concourse.bacc as bacc
import concourse.mybir as mybir
import concourse.tile as tile
from concourse.bass import ds
from concourse.bass_utils import run_bass_kernel_spmd

F32 = mybir.dt.float32

N = 2048            # sequence length (both source and target)
P = 128             # partitions / column chunks
CW = N // P         # columns per chunk (16)
SW = 2 * CW + 2     # strip width: [halo | 2*CW scan slots | pad]
SLACK = 2           # wavefront steps of slack per chunk
T = N + SLACK * (P - 1)   # 2302 total wavefront steps
PH1 = 1152                # phase-1 steps (even)
PH2 = T - PH1             # phase-2 steps (1150, even)
M = T + 2                 # sdiag columns (padded)
PADF = 2 * (P - 1) + 2    # front padding of s_pad (256)
SPAD = PADF + M           # s_pad length
INF = np.float32(1e30)
PAD = np.float32(1e15)    # sdiag pad; squares to 1e30

_cache = {}


def _build():
    nc = bacc.Bacc("TRN2", target_bir_lowering=False, debug=False)

    s_pad = nc.dram_tensor("s_pad", [1, SPAD], F32, kind="ExternalInput")
    negt = nc.dram_tensor("negt", [P, CW], F32, kind="ExternalInput")
    res = nc.dram_tensor("res", [1, 1], F32, kind="ExternalOutput")

    with tile.TileContext(nc) as tc:
        with tc.tile_pool(name="sb", bufs=1) as pool:
            t_sdiag = pool.tile([P, M], F32)
            t_negt = pool.tile([P, CW], F32)
            t_ring = pool.tile([P, PH1 * 2 * CW], F32)
            t_sa = pool.tile([P, SW], F32)
            t_sb = pool.tile([P, SW], F32)

            # Partition p owns column block 127-p (delay 2*(127-p)):
            # sdiag[p, m] = s_pad[PADF - 2*(P-1) + 2p + m] = s[m - 2*(127-p)]
            sd_src = bass.AP(s_pad, PADF - SLACK * (P - 1), [[SLACK, P], [1, M]])
            nc.sync.dma_start(t_sdiag[:], sd_src)
            nc.sync.dma_start(t_negt[:], negt[:])

            # zeros in the even (d1) slots persist for the whole run
            nc.gpsimd.memset(t_ring[:], 0.0)
            nc.vector.memset(t_sa[:], float(INF))
            nc.vector.memset(t_sb[:], float(INF))

            strips = [t_sa, t_sb]
            eng = nc.vector
            pstr = int(t_sa.ap[0][0])
            rstr = int(t_ring.ap[0][0])

            def produce(phase_base, cnt):
                # ring[p, step*2CW + 2j+1] = (sdiag[p, phase_base+step] - t_j)^2
                for j in range(CW):
                    out_ap = bass.AP(
                        t_ring.tensor,
                        t_ring.offset + 2 * j + 1,
                        [[rstr, P], [2 * CW, cnt]],
                    )
                    nc.scalar.activation(
                        out_ap,
                        t_sdiag[:, phase_base : phase_base + cnt],
                        mybir.ActivationFunctionType.Square,
                        bias=t_negt[:, j : j + 1],
                        scale=1.0,
                    )

            def scan_step(cur, prev, d1_ap):
                d0 = bass.AP(
                    prev.tensor, prev.offset + 2, [[pstr, P], [2, CW], [-2, 2]]
                )
                eng.add_instruction(
                    mybir.InstTensorScalarPtr(
                        name=nc.get_next_instruction_name(),
                        is_tensor_tensor_scan=True,
                        is_scalar_tensor_tensor=True,
                        op0=mybir.AluOpType.min,
                        op1=mybir.AluOpType.add,
                        ins=[
                            eng.lower_ap(d0),
                            eng.lower_ap(cur[:, 0:1]),
                            eng.lower_ap(d1_ap),
                        ],
                        outs=[eng.lower_ap(cur[:, 1 : 2 * CW + 1])],
                    )
                )

            def halo_dma(strip):
                # block b's halo comes from block b-1 = partition p+1:
                # strip[p+1, 2CW] -> strip[p, 0] for p = 0..P-2
                nc.sync.dma_start(
                    strip[0 : P - 1, 0:1], strip[1:P, 2 * CW : 2 * CW + 1]
                )

            def phase(nsteps):
                # The halo from scan t is shipped during step t+1 (one step of
                # delay), so it lands after scan t+1 consumed the previous halo
                # as its upleft value and before scan t+2 reads it as initial.
                with tc.For_i(0, nsteps // 2, 1) as i:
                    scan_step(t_sa, t_sb, t_ring[:, ds(i * (4 * CW), 2 * CW)])
                    halo_dma(t_sb)
                    scan_step(
                        t_sb, t_sa, t_ring[:, ds(i * (4 * CW) + 2 * CW, 2 * CW)]
                    )
                    halo_dma(t_sa)

            # ---- phase 1: steps 0 .. PH1-1
            produce(0, PH1)
            # DP corner: DTW[0,0]=0 enters via ring slot (t=0, cell 0, even):
            # state = min(INF, INF) + (-1e30) = 0, then the odd slot adds the
            # real cost. Broadcast to all partitions (single-partition APs at
            # p=127 are rejected): on blocks b>=1 step 0 is a garbage row
            # whose ~1e30 cost re-swamps the zeroed state, so it's harmless.
            nc.vector.memset(t_ring[:, 0:1], -float(INF))
            phase(PH1)

            # ---- phase 2: steps PH1 .. T-1
            nc.vector.memset(t_ring[:, 0:1], 0.0)
            produce(PH1, PH2)
            phase(PH2)

            # final DP cell: block 127 = partition 0, at step T-1
            final = strips[(T - 1) % 2]
            nc.sync.dma_start(res[0:1, 0:1], final[0:1, 2 * CW : 2 * CW + 1])
    # Declare a custom-DVE op so compile_bir_kernel builds a per-NEFF DVE
    # table via dve_table_for_ops (cached process-wide); otherwise every call
    # regenerates the stock default table (~50ms/call). The table is the
    # stock base plus this op, so the scan's stock uOps are unaffected.
    nc.m.ant_custom_dve_ops = ["ADD_RANGE_WRAP"]
    nc.compile()
    return nc


def _prep_inputs(source, target):
    source = np.asarray(source, np.float32).reshape(N)
    target = np.asarray(target, np.float32).reshape(N)
    sp = np.full((1, SPAD), PAD, np.float32)
    sp[0, PADF : PADF + N] = source
    # partition p owns column block 127-p
    negt = (-target.reshape(P, CW)[::-1]).astype(np.float32).copy()
    return {"s_pad": sp, "negt": negt}


def _run(inputs, trace=False):
    if "nc" not in _cache:
        _cache["nc"] = _build()
    nc = _cache["nc"]
    r = run_bass_kernel_spmd(
        nc, [dict(inputs) for _ in range(8)], core_ids=list(range(8)), trace=trace
    )
    return r


def kernel(source, target):
    inputs = _prep_inputs(source, target)
    r = _run(inputs)
    loss_sq = r.results[0]["res"][0, 0]
    return np.sqrt(np.float32(loss_sq))[None].astype(np.float32)
